# revision 1
# baseline (speedup 1.0000x reference)
"""Trainium2 Bass kernel for nn_Conv2d_NN (retrieval_knn).

Reference computation (per batch b):
  xf = x.reshape(B, C, T)                       # T = H*W = 4096, C = 32
  xn = xf / ||xf||_2(channel axis)              # cosine-normalize tokens
  sim = clip(xn^T xn, -1, 1)                    # [T, T]
  vals, idx = top_k(sim, 9)                     # per row, sorted desc
  prime[c,t,k] = vals[t,k] * xf[c, idx[t,k]]
  out[o,t] = sum_{c,k} prime[c,t,k] * w[o,c,k] + bias[o]

Sharding: data-parallel over batch, one batch per NeuronCore (8 cores).

Per-core device algorithm (flash-style fused top-k, sim never hits HBM):
  stage 1: per-token inverse norms via PE transposes + ACT square-accum;
           normalized xn replicated onto all 4 PE row-groups [128, T].
  stage 2, per super-block of 4 row blocks (4 x 128 tokens):
    - sim row blocks via 4-way row-group-packed fp32 matmuls (K=32 each)
    - ACT evicts PSUM -> SBUF sim rows, gpsimd masks the diagonal
    - DVE max / max_index -> top-8 values + columns per row
    - slot 0 = self (val 1.0, idx = row token): top-9 assembled
    - gpsimd ap_gather pulls all 4*9*128 neighbor feature columns from the
      raw x [32, 4096] SBUF tile (indices shared across channel partitions)
    - columns scaled by vals (partition-broadcast row), conv contraction =
      9 accumulating [32x32]x[32x512] fp16 matmuls + bias.

Gather column order: j = (q*36 + r*9 + k)*16 + pp where the token is
p = pp*8 + q of row block r (q in [0,8), pp in [0,16)) and k is the
neighbor slot.  This is ap_gather's natural wrapped index order, builds
from idx16 [128, 36] with one DMA per 16-partition replica, and keeps
each k-slice of the gathered matrix an affine matmul access pattern
whose walk order is exactly super-block token order.
"""

import sys

if "/opt/trn_rl_repo" not in sys.path:
    sys.path.insert(0, "/opt/trn_rl_repo")

import numpy as np

B, C, H, W = 8, 32, 64, 64
T = H * W          # 4096
KNN = 9            # neighbors
NCORES = 8
RBS = 128          # row-block size (tokens per block)
NRB = T // RBS     # 32
SUP = 4            # row blocks per super-block
NSUP = NRB // SUP  # 8
SBS = SUP * RBS    # 512 tokens per super-block
CBS = 512          # col-block size (matmul moving dim)
NCB = T // CBS     # 8
O = 32             # conv output channels
RK = SUP * KNN     # 36 (row-block, k) pairs per token-slot group
NI = RBS * RK      # 4608 gathered columns per super-block

_CACHE = {}


def _build_program(debug_outs=False):
    import concourse.bass as bass
    import concourse.bacc as bacc
    import concourse.mybir as mybir
    from concourse.tile import TileContext
    from concourse.masks import make_identity

    f32 = mybir.dt.float32
    i16 = mybir.dt.int16
    u16 = mybir.dt.uint16
    f16 = mybir.dt.float16

    nc = bacc.Bacc("TRN2", target_bir_lowering=False, debug=False,
                   num_devices=NCORES)

    xb = nc.dram_tensor("xb", [C, T], f32, kind="ExternalInput")
    wf = nc.dram_tensor("wf", [KNN * C, O], f16, kind="ExternalInput")
    bias = nc.dram_tensor("bias", [O, 1], f32, kind="ExternalInput")
    out = nc.dram_tensor("out", [O, T], f32, kind="ExternalOutput")
    if debug_outs:
        xn_d = nc.dram_tensor("xn_d", [C, T], f32, kind="ExternalOutput")
        vals_d = nc.dram_tensor("vals_d", [NRB, RBS, KNN], f32,
                                kind="ExternalOutput")
        idx_d = nc.dram_tensor("idx_d", [NRB, RBS, KNN], i16,
                               kind="ExternalOutput")
        g_d = nc.dram_tensor("g_d", [NSUP, C, NI], f32,
                             kind="ExternalOutput")

    AF = mybir.ActivationFunctionType
    ALU = mybir.AluOpType

    with TileContext(nc) as tc:
        with (
            tc.tile_pool(name="const", bufs=1) as cpool,
            tc.tile_pool(name="xdata", bufs=1) as xpool,
        ):
            ident128 = cpool.tile([128, 128], f32)
            make_identity(nc, ident128[:])
            ident32 = cpool.tile([32, 32], f32)
            make_identity(nc, ident32[:])
            # rep4[c, m] = 1 iff m % 32 == c: replicates [32, N] onto all
            # four 32-partition row groups via one exact matmul
            rep4 = cpool.tile([C, 128], f32)
            for g in range(4):
                nc.vector.tensor_copy(rep4[:, 32 * g:32 * (g + 1)],
                                      ident32[:])
            # iota4[p, r] = p + r*128 (token id of partition p in row blk r)
            iota4 = cpool.tile([128, SUP], u16)
            nc.gpsimd.iota(iota4[:], pattern=[[RBS, SUP]], base=0,
                           channel_multiplier=1)
            wf_sb = []
            for k in range(KNN):
                wf_k = cpool.tile([C, O], f16, name=f"wf_k{k}")
                nc.sync.dma_start(out=wf_k[:],
                                  in_=wf.ap()[k * C:(k + 1) * C, :])
                wf_sb.append(wf_k)
            ones16 = cpool.tile([1, C], f16)
            nc.gpsimd.memset(ones16[:], 1.0)
            bias_sb = cpool.tile([O, 1], f32)
            nc.sync.dma_start(out=bias_sb[:], in_=bias.ap())

            # raw x replicated onto all four 32-partition row groups
            # (gather source for indirect_copy, which needs 128 partitions)
            xb_rep = xpool.tile([128, T], f32)
            nc.sync.dma_start(
                out=xb_rep[:],
                in_=xb.ap().to_broadcast([C, T, 4]).rearrange(
                    "c t g -> g c t"))
            # xn replicated onto all four 32-partition row groups
            xn_rep = xpool.tile([128, T], f32)

            # ---- stage 1: inverse norms, normalized + replicated xn ----
            with (
                tc.tile_pool(name="s1ps", bufs=2, space="PSUM") as s1ps,
                tc.tile_pool(name="s1sb", bufs=3) as s1sb,
            ):
                for blk in range(NRB):
                    cs = slice(blk * RBS, (blk + 1) * RBS)
                    tp = s1ps.tile([RBS, C], f32, tag="tp")
                    nc.tensor.matmul(tp[:], lhsT=xb_rep[0:C, cs],
                                     rhs=ident32[:], is_transpose=True)
                    xT_blk = s1sb.tile([RBS, C], f32, tag="xT_blk")
                    nc.scalar.activation(xT_blk[:], tp[:], AF.Copy)
                    sq = s1sb.tile([RBS, C], f32, tag="sq")
                    nsq = s1sb.tile([RBS, 1], f32, tag="nsq")
                    nc.scalar.activation(sq[:], xT_blk[:], AF.Square,
                                         accum_out=nsq[:])
                    nrm = s1sb.tile([RBS, 1], f32, tag="nrm")
                    nc.scalar.activation(nrm[:], nsq[:], AF.Sqrt)
                    rinv = s1sb.tile([RBS, 1], f32, tag="rinv")
                    nc.vector.reciprocal(rinv[:], nrm[:])
                    xnT_blk = s1sb.tile([RBS, C], f32, tag="xnT_blk")
                    nc.vector.tensor_scalar_mul(xnT_blk[:], xT_blk[:], rinv[:])
                    # transpose back, then replicate onto all 4 row groups
                    tp2 = s1ps.tile([C, RBS], f32, tag="tp2")
                    nc.tensor.matmul(tp2[:], lhsT=xnT_blk[:],
                                     rhs=ident128[:], is_transpose=True)
                    xn_blk = s1sb.tile([C, RBS], f32, tag="xn_blk")
                    nc.scalar.activation(xn_blk[:], tp2[:], AF.Copy)
                    tp3 = s1ps.tile([128, RBS], f32, tag="tp3")
                    nc.tensor.matmul(tp3[:], lhsT=rep4[:], rhs=xn_blk[:],
                                     start=True, stop=True)
                    nc.scalar.activation(xn_rep[:, cs], tp3[:], AF.Copy)

            # ---- stage 2: fused sim + top-k + gather + conv ----
            tc.strict_bb_all_engine_barrier()
            with (
                tc.tile_pool(name="simps", bufs=2, space="PSUM") as simps,
                tc.tile_pool(name="ops", bufs=1, space="PSUM") as ops,
                tc.tile_pool(name="vps", bufs=2, space="PSUM") as vps,
                tc.tile_pool(name="row", bufs=2) as rowpool,
                tc.tile_pool(name="small", bufs=4) as spool,
                tc.tile_pool(name="big", bufs=2) as bpool,
            ):
                def emit_out_stage(psb, vals9, idx16):
                        # ---- wrapped index tile for ap_gather ----
                        idxw = spool.tile([32, NI // 16], i16, tag="idxw")
                        for gr in range(2):
                            nc.gpsimd.dma_start(
                                out=idxw[gr * 16:(gr + 1) * 16, :].rearrange(
                                    "pp (q rk) -> pp q rk", q=8),
                                in_=idx16[:].bitcast(i16))
                        # vals row (fp16): vrow[0, p*36 + rk] = vals9[p, rk]
                        vals9h = spool.tile([RBS, RK], f16, tag="vals9h")
                        nc.vector.tensor_copy(vals9h[:], vals9[:])
                        vrow = bpool.tile([1, NI], f16, tag="vrow")
                        nc.gpsimd.dma_start(out=vrow[:], in_=vals9h[:])
                        # j-order view of the p-major row
                        vrowj = vrow[:].rearrange("one (pp q rk) -> one q rk pp",
                                                  pp=16, q=8)
                        # ---- gather + scale + contract ----
                        gg = bpool.tile([C, NI], f32, tag="gg")
                        nc.gpsimd.ap_gather(
                            out_ap=gg[:].rearrange("p (n d) -> p n d", d=1),
                            in_ap=xb_rep[0:C, :].rearrange("p (n d) -> p n d",
                                                           d=1),
                            idxs_ap=idxw[:],
                            channels=32, num_elems=T, d=1, num_idxs=NI)
                        # scale: 16 chunks of 288 j-columns; valsb broadcast
                        # into PSUM via fp16 ones-matmul, multiply on DVE
                        pp_t = bpool.tile([C, NI], f16, tag="pp_t")
                        CH = 288
                        for c in range(NI // CH):
                            q, rh = divmod(c, 2)
                            vb_ps = vps.tile([C, CH], f32, tag="vb_ps",
                                             name="vb_ps")
                            nc.tensor.matmul(
                                vb_ps[:], lhsT=ones16[:],
                                rhs=vrowj[:, q, rh * 18:(rh + 1) * 18, :],
                                start=True, stop=True)
                            nc.vector.tensor_tensor(
                                out=pp_t[:, c * CH:(c + 1) * CH],
                                in0=gg[:, c * CH:(c + 1) * CH],
                                in1=vb_ps[:], op=ALU.mult)
                        out_ps = ops.tile([O, SBS], f32, tag="out_ps")
                        # per-k view, walk (r, pp, q) == super-block token order
                        pview = pp_t[:].rearrange(
                            "c (q r k pp) -> c k r pp q", q=8, r=SUP, k=KNN)
                        for k in range(KNN):
                            nc.tensor.matmul(out_ps[:], lhsT=wf_sb[k][:],
                                             rhs=pview[:, k],
                                             start=(k == 0), stop=(k == KNN - 1))
                        out_sb = spool.tile([O, SBS], f32, tag="out_sb")
                        nc.scalar.activation(out_sb[:], out_ps[:], AF.Identity,
                                             bias=bias_sb[:])
                        nc.scalar.dma_start(
                            out=out.ap()[:, psb * SBS:(psb + 1) * SBS],
                            in_=out_sb[:])

                pending = []
                for sb in range(NSUP):
                    vals9 = spool.tile([RBS, RK], f32, tag="vals9")
                    idx16 = spool.tile([RBS, RK], u16, tag="idx16")
                    v3 = vals9[:].rearrange("p (r k) -> p r k", r=SUP)
                    i3 = idx16[:].rearrange("p (r k) -> p r k", r=SUP)
                    nc.gpsimd.memset(v3[:, :, 0:1], 1.0)
                    nc.gpsimd.tensor_scalar_add(
                        i3[:, :, 0:1],
                        iota4[:].rearrange("p (r one) -> p r one", one=1),
                        sb * SBS)
                    for r in range(SUP):
                        rb = sb * SUP + r
                        rs = slice(rb * RBS, (rb + 1) * RBS)
                        simrow = rowpool.tile([RBS, T], f32, tag="simrow")
                        # 2 quads of 4-way row-group-packed fp32 matmuls;
                        # each [128,1024] psum tile holds 2 col blocks
                        for half in range(2):
                            for j in range(2):
                                ps = simps.tile([RBS, 2 * CBS], f32,
                                                tag="ps", name="ps")
                                for gi in range(2):
                                    g = 2 * j + gi
                                    cb = half * 4 + 2 * j + gi
                                    cs2 = slice(cb * CBS, (cb + 1) * CBS)
                                    nc.tensor.matmul(
                                        ps[:, gi * CBS:(gi + 1) * CBS],
                                        lhsT=xn_rep[32 * g:32 * (g + 1), rs],
                                        rhs=xn_rep[32 * g:32 * (g + 1), cs2],
                                        tile_position=(32 * g, 0),
                                        start=True, stop=True,
                                        skip_group_check=True)
                                c0 = (half * 4 + 2 * j) * CBS
                                nc.scalar.activation(
                                    simrow[:, c0:c0 + 2 * CBS], ps[:],
                                    AF.Copy)
                        # mask self-similarity to -2
                        nc.gpsimd.affine_select(
                            out=simrow[:, rs], in_=simrow[:, rs],
                            pattern=[[-1, RBS]], channel_multiplier=1, base=0,
                            compare_op=ALU.not_equal, fill=-2.0)
                        nc.vector.max(out=v3[:, r, 1:KNN], in_=simrow[:])
                        nc.vector.max_index(
                            out=i3[:, r, 1:KNN],
                            in_max=v3[:, r, 1:KNN], in_values=simrow[:])
                    pending.append((sb, vals9, idx16))
                    if len(pending) > 1:
                        emit_out_stage(*pending.pop(0))
                    if debug_outs:
                        for r in range(SUP):
                            rb = psb * SUP + r
                            nc.sync.dma_start(out=vals_d.ap()[rb],
                                              in_=v3[:, r])
                            nc.sync.dma_start(out=idx_d.ap()[rb],
                                              in_=i3[:, r])
                        nc.sync.dma_start(out=g_d.ap()[psb], in_=gg[0:C, :])
                for st in pending:
                    emit_out_stage(*st)
                if debug_outs:
                    nc.sync.dma_start(out=xn_d.ap(), in_=xn_rep[0:32, :])
    nc.compile()
    return nc


def _get_program():
    if "nc" not in _CACHE:
        _CACHE["nc"] = _build_program()
    return _CACHE["nc"]


def _prep_inputs(x, weight, bias):
    xf = np.ascontiguousarray(np.asarray(x, dtype=np.float32).reshape(B, C, T))
    # wf[(k,c), o] = weight[o, c, k]
    wfm = np.ascontiguousarray(
        np.asarray(weight, dtype=np.float32).transpose(2, 1, 0).reshape(
            KNN * C, O).astype(np.float16))
    bp = np.ascontiguousarray(np.asarray(bias, dtype=np.float32).reshape(O, 1))
    return [
        {"xb": np.ascontiguousarray(xf[b]), "wf": wfm, "bias": bp}
        for b in range(B)
    ]


def kernel(x, weight, bias):
    from concourse import bass_utils

    nc = _get_program()
    in_maps = _prep_inputs(x, weight, bias)
    res = bass_utils.run_bass_kernel_spmd(nc, in_maps,
                                          core_ids=list(range(NCORES)))
    out = np.stack([res.results[b]["out"] for b in range(B)])
    return np.ascontiguousarray(out.reshape(B, O, H, W).astype(np.float32))



# revision 24
# speedup vs baseline: 1.0331x; 1.0331x over previous
"""Trainium2 Bass kernel for nn_Conv2d_NN (retrieval_knn).

Reference computation (per batch b):
  xf = x.reshape(B, C, T)                       # T = H*W = 4096, C = 32
  xn = xf / ||xf||_2(channel axis)              # cosine-normalize tokens
  sim = clip(xn^T xn, -1, 1)                    # [T, T]
  vals, idx = top_k(sim, 9)                     # per row, sorted desc
  prime[c,t,k] = vals[t,k] * xf[c, idx[t,k]]
  out[o,t] = sum_{c,k} prime[c,t,k] * w[o,c,k] + bias[o]

Sharding: data-parallel over batch, one batch per NeuronCore (8 cores).

Per-core device algorithm (flash-style fused top-k, sim never hits HBM),
same instruction vocabulary as the original baseline but restructured for
pipeline overlap: the per-super-block tail is split into three stages
emitted at pipeline depths 0/1/2 so no engine FIFO head ever blocks on a
result that is still being produced:

  rb phase (depth 0): sim row blocks via 4-way row-group-packed fp32
    matmuls, ACT eviction, gpsimd diag masking, DVE max8/find_index8.
  stage A (depth 0, after top-k): vals cast, index-tile DMAs, vals-row
    DMA, gpsimd ap_gather of neighbor columns.
  stage M (depth 1): vals broadcast matmuls (PE) + scale multiplies (DVE)
    - their inputs finished during the previous super-block.
  stage B (depth 2): conv contraction + bias + output DMA - the scaled
    input finished during the previous super-block, so the Tensor queue
    never stalls ahead of the next sim matmuls.

Gather column order: j = (q*36 + r*9 + k)*16 + pp where the token is
p = pp*8 + q of row block r (q in [0,8), pp in [0,16)) and k is the
neighbor slot.
"""

import sys

if "/opt/trn_rl_repo" not in sys.path:
    sys.path.insert(0, "/opt/trn_rl_repo")

import numpy as np

B, C, H, W = 8, 32, 64, 64
T = H * W          # 4096
KNN = 9            # neighbors
NCORES = 8
RBS = 128          # row-block size (tokens per block)
NRB = T // RBS     # 32
SUP = 4            # row blocks per super-block
NSUP = NRB // SUP  # 8
SBS = SUP * RBS    # 512 tokens per super-block
CBS = 512          # col-block size (matmul moving dim)
NCB = T // CBS     # 8
O = 32             # conv output channels
RK = SUP * KNN     # 36 (row-block, k) pairs per token-slot group
NI = RBS * RK      # 4608 gathered columns per super-block

_CACHE = {}


def _build_program():
    import concourse.bass as bass
    import concourse.bacc as bacc
    import concourse.mybir as mybir
    from concourse.tile import TileContext
    from concourse.masks import make_identity

    f32 = mybir.dt.float32
    i16 = mybir.dt.int16
    u16 = mybir.dt.uint16
    f16 = mybir.dt.float16

    nc = bacc.Bacc("TRN2", target_bir_lowering=False, debug=False,
                   num_devices=NCORES)

    xb = nc.dram_tensor("xb", [C, T], f32, kind="ExternalInput")
    wf = nc.dram_tensor("wf", [KNN * C, O], f16, kind="ExternalInput")
    bias = nc.dram_tensor("bias", [O, 1], f32, kind="ExternalInput")
    out = nc.dram_tensor("out", [O, T], f32, kind="ExternalOutput")

    AF = mybir.ActivationFunctionType
    ALU = mybir.AluOpType

    with TileContext(nc) as tc:
        with (
            tc.tile_pool(name="const", bufs=1) as cpool,
            tc.tile_pool(name="xdata", bufs=1) as xpool,
        ):
            ident128 = cpool.tile([128, 128], f32)
            make_identity(nc, ident128[:])
            ident32 = cpool.tile([32, 32], f32)
            make_identity(nc, ident32[:])
            # rep4[c, m] = 1 iff m % 32 == c: replicates [32, N] onto all
            # four 32-partition row groups via one exact matmul
            rep4 = cpool.tile([C, 128], f32)
            for g in range(4):
                nc.vector.tensor_copy(rep4[:, 32 * g:32 * (g + 1)],
                                      ident32[:])
            # iota4[p, r] = p + r*128 (token id of partition p in row blk r)
            iota4 = cpool.tile([128, SUP], u16)
            nc.gpsimd.iota(iota4[:], pattern=[[RBS, SUP]], base=0,
                           channel_multiplier=1)
            wf_sb = []
            for k in range(KNN):
                wf_k = cpool.tile([C, O], f16, name=f"wf_k{k}")
                nc.sync.dma_start(out=wf_k[:],
                                  in_=wf.ap()[k * C:(k + 1) * C, :])
                wf_sb.append(wf_k)
            ones16 = cpool.tile([1, C], f16)
            nc.gpsimd.memset(ones16[:], 1.0)
            bias_sb = cpool.tile([O, 1], f32)
            nc.sync.dma_start(out=bias_sb[:], in_=bias.ap())

            # raw x replicated onto all four 32-partition row groups
            # (gather source for indirect_copy, which needs 128 partitions)
            xb_rep = xpool.tile([128, T], f32)
            nc.sync.dma_start(
                out=xb_rep[:],
                in_=xb.ap().to_broadcast([C, T, 4]).rearrange(
                    "c t g -> g c t"))
            # xn replicated onto all four 32-partition row groups
            xn_rep = xpool.tile([128, T], f32)

            # ---- stage 1: inverse norms, normalized + replicated xn ----
            with (
                tc.tile_pool(name="s1ps", bufs=2, space="PSUM") as s1ps,
                tc.tile_pool(name="s1sb", bufs=3) as s1sb,
            ):
                for blk in range(NRB):
                    cs = slice(blk * RBS, (blk + 1) * RBS)
                    tp = s1ps.tile([RBS, C], f32, tag="tp")
                    nc.tensor.matmul(tp[:], lhsT=xb_rep[0:C, cs],
                                     rhs=ident32[:], is_transpose=True)
                    xT_blk = s1sb.tile([RBS, C], f32, tag="xT_blk")
                    nc.scalar.activation(xT_blk[:], tp[:], AF.Copy)
                    sq = s1sb.tile([RBS, C], f32, tag="sq")
                    nsq = s1sb.tile([RBS, 1], f32, tag="nsq")
                    nc.scalar.activation(sq[:], xT_blk[:], AF.Square,
                                         accum_out=nsq[:])
                    nrm = s1sb.tile([RBS, 1], f32, tag="nrm")
                    nc.scalar.activation(nrm[:], nsq[:], AF.Sqrt)
                    rinv = s1sb.tile([RBS, 1], f32, tag="rinv")
                    nc.vector.reciprocal(rinv[:], nrm[:])
                    xnT_blk = s1sb.tile([RBS, C], f32, tag="xnT_blk")
                    nc.vector.tensor_scalar_mul(xnT_blk[:], xT_blk[:], rinv[:])
                    # transpose back, then replicate onto all 4 row groups
                    tp2 = s1ps.tile([C, RBS], f32, tag="tp2")
                    nc.tensor.matmul(tp2[:], lhsT=xnT_blk[:],
                                     rhs=ident128[:], is_transpose=True)
                    xn_blk = s1sb.tile([C, RBS], f32, tag="xn_blk")
                    nc.scalar.activation(xn_blk[:], tp2[:], AF.Copy)
                    tp3 = s1ps.tile([128, RBS], f32, tag="tp3")
                    nc.tensor.matmul(tp3[:], lhsT=rep4[:], rhs=xn_blk[:],
                                     start=True, stop=True)
                    nc.scalar.activation(xn_rep[:, cs], tp3[:], AF.Copy)

            # ---- stage 2: fused sim + top-k + gather + conv ----
            tc.strict_bb_all_engine_barrier()
            with (
                tc.tile_pool(name="simps", bufs=2, space="PSUM") as simps,
                tc.tile_pool(name="ops", bufs=1, space="PSUM") as ops,
                tc.tile_pool(name="vps", bufs=2, space="PSUM") as vps,
                tc.tile_pool(name="row", bufs=3) as rowpool,
                tc.tile_pool(name="small", bufs=4) as spool,
                tc.tile_pool(name="big", bufs=2) as bpool,
            ):
                def emit_stage_a(psb, vals9, idx16):
                    # ---- wrapped index tile for ap_gather ----
                    idxw = spool.tile([32, NI // 16], i16, tag="idxw")
                    for gr in range(2):
                        nc.gpsimd.dma_start(
                            out=idxw[gr * 16:(gr + 1) * 16, :].rearrange(
                                "pp (q rk) -> pp q rk", q=8),
                            in_=idx16[:].bitcast(i16))
                    # vals row (fp16): vrow[0, p*36 + rk] = vals9[p, rk]
                    vals9h = spool.tile([RBS, RK], f16, tag="vals9h")
                    nc.vector.tensor_copy(vals9h[:], vals9[:])
                    vrow = bpool.tile([1, NI], f16, tag="vrow")
                    nc.gpsimd.dma_start(out=vrow[:], in_=vals9h[:])
                    # ---- gather ----
                    gg = bpool.tile([C, NI], f32, tag="gg")
                    nc.gpsimd.ap_gather(
                        out_ap=gg[:].rearrange("p (n d) -> p n d", d=1),
                        in_ap=xb_rep[0:C, :].rearrange("p (n d) -> p n d",
                                                       d=1),
                        idxs_ap=idxw[:],
                        channels=32, num_elems=T, d=1, num_idxs=NI)
                    return psb, vrow, gg

                def emit_stage_m(psb, vrow, gg):
                    # j-order view of the p-major row
                    vrowj = vrow[:].rearrange("one (pp q rk) -> one q rk pp",
                                              pp=16, q=8)
                    # scale: 16 chunks of 288 j-columns; valsb broadcast
                    # into PSUM via fp16 ones-matmul, multiply on DVE
                    pp_t = bpool.tile([C, NI], f16, tag="pp_t")
                    CH = 288
                    for c in range(NI // CH):
                        q, rh = divmod(c, 2)
                        vb_ps = vps.tile([C, CH], f32, tag="vb_ps",
                                         name="vb_ps")
                        nc.tensor.matmul(
                            vb_ps[:], lhsT=ones16[:],
                            rhs=vrowj[:, q, rh * 18:(rh + 1) * 18, :],
                            start=True, stop=True)
                        nc.vector.tensor_tensor(
                            out=pp_t[:, c * CH:(c + 1) * CH],
                            in0=gg[:, c * CH:(c + 1) * CH],
                            in1=vb_ps[:], op=ALU.mult)
                    return psb, pp_t

                def emit_stage_b(psb, pp_t):
                    out_ps = ops.tile([O, SBS], f32, tag="out_ps")
                    # per-k view, walk (r, pp, q) == super-block token order
                    pview = pp_t[:].rearrange(
                        "c (q r k pp) -> c k r pp q", q=8, r=SUP, k=KNN)
                    for k in range(KNN):
                        nc.tensor.matmul(out_ps[:], lhsT=wf_sb[k][:],
                                         rhs=pview[:, k],
                                         start=(k == 0), stop=(k == KNN - 1))
                    out_sb = spool.tile([O, SBS], f32, tag="out_sb")
                    nc.scalar.activation(out_sb[:], out_ps[:], AF.Identity,
                                         bias=bias_sb[:])
                    nc.sync.dma_start(
                        out=out.ap()[:, psb * SBS:(psb + 1) * SBS],
                        in_=out_sb[:])

                pending_m = []
                pending_b = []
                for sb in range(NSUP):
                    if pending_b:
                        emit_stage_b(*pending_b.pop(0))
                    vals9 = spool.tile([RBS, RK], f32, tag="vals9")
                    idx16 = spool.tile([RBS, RK], u16, tag="idx16")
                    v3 = vals9[:].rearrange("p (r k) -> p r k", r=SUP)
                    i3 = idx16[:].rearrange("p (r k) -> p r k", r=SUP)
                    nc.gpsimd.memset(v3[:, :, 0:1], 1.0)
                    nc.gpsimd.tensor_scalar_add(
                        i3[:, :, 0:1],
                        iota4[:].rearrange("p (r one) -> p r one", one=1),
                        sb * SBS)
                    for r in range(SUP):
                        rb = sb * SUP + r
                        rs = slice(rb * RBS, (rb + 1) * RBS)
                        simrow = rowpool.tile([RBS, T], f32, tag="simrow")
                        # 2 quads of 4-way row-group-packed fp32 matmuls;
                        # each [128,1024] psum tile holds 2 col blocks
                        for half in range(2):
                            for j in range(2):
                                ps = simps.tile([RBS, 2 * CBS], f32,
                                                tag="ps", name="ps")
                                for gi in range(2):
                                    g = 2 * j + gi
                                    cb = half * 4 + 2 * j + gi
                                    cs2 = slice(cb * CBS, (cb + 1) * CBS)
                                    nc.tensor.matmul(
                                        ps[:, gi * CBS:(gi + 1) * CBS],
                                        lhsT=xn_rep[32 * g:32 * (g + 1), rs],
                                        rhs=xn_rep[32 * g:32 * (g + 1), cs2],
                                        tile_position=(32 * g, 0),
                                        start=True, stop=True,
                                        skip_group_check=True)
                                c0 = (half * 4 + 2 * j) * CBS
                                nc.scalar.activation(
                                    simrow[:, c0:c0 + 2 * CBS], ps[:],
                                    AF.Copy)
                        # mask self-similarity to -2
                        nc.gpsimd.affine_select(
                            out=simrow[:, rs], in_=simrow[:, rs],
                            pattern=[[-1, RBS]], channel_multiplier=1, base=0,
                            compare_op=ALU.not_equal, fill=-2.0)
                        nc.vector.max(out=v3[:, r, 1:KNN], in_=simrow[:])
                        nc.vector.max_index(
                            out=i3[:, r, 1:KNN],
                            in_max=v3[:, r, 1:KNN], in_values=simrow[:])
                    pending_a = emit_stage_a(sb, vals9, idx16)
                    if pending_m:
                        pending_b.append(emit_stage_m(*pending_m.pop(0)))
                    pending_m.append(pending_a)
                for st in pending_m:
                    pending_b.append(emit_stage_m(*st))
                for st in pending_b:
                    emit_stage_b(*st)
    nc.compile()
    return nc


def _get_program():
    if "nc" not in _CACHE:
        _CACHE["nc"] = _build_program()
    return _CACHE["nc"]


def _prep_inputs(x, weight, bias):
    xf = np.ascontiguousarray(np.asarray(x, dtype=np.float32).reshape(B, C, T))
    # wf[(k,c), o] = weight[o, c, k]
    wfm = np.ascontiguousarray(
        np.asarray(weight, dtype=np.float32).transpose(2, 1, 0).reshape(
            KNN * C, O).astype(np.float16))
    bp = np.ascontiguousarray(np.asarray(bias, dtype=np.float32).reshape(O, 1))
    return [
        {"xb": np.ascontiguousarray(xf[b]), "wf": wfm, "bias": bp}
        for b in range(B)
    ]


def kernel(x, weight, bias):
    from concourse import bass_utils

    nc = _get_program()
    in_maps = _prep_inputs(x, weight, bias)
    res = bass_utils.run_bass_kernel_spmd(nc, in_maps,
                                          core_ids=list(range(NCORES)))
    out = np.stack([res.results[b]["out"] for b in range(B)])
    return np.ascontiguousarray(out.reshape(B, O, H, W).astype(np.float32))
